# revision 21
# baseline (speedup 1.0000x reference)
"""BasicConvClassifier on 8 Trainium2 NeuronCores.

Strategy (quarter-subject sharding with A/B stream interleave):
  - Sort the batch by subject (4 subjects). The two largest subjects become
    "A-stream" subjects, the two smallest "B-stream". Cores 0-3 share the
    quarters of A-subject#0 and B-subject#0; cores 4-7 share A-subject#1 /
    B-subject#1. Each core holds KA A-samples and KB B-samples (padded with
    zero samples to uniform KA/KB, multiple of 8).
  - Per-subject BN stats reduce over a 4-core AllGather group. Each layer
    issues TWO collectives (A then B); the 15us collective latency of stream
    A is hidden under stream B's conv/stats work and vice versa, so the
    per-layer sync cost mostly disappears from the critical path.
  - Pad samples follow a dedicated reference-pad slot (X = 0) per stream;
    their stats contribution is subtracted exactly as npad * ref.
  - Conv1d(k=3, SAME) is shifted fp32r matmuls accumulated in PSUM; the
    residual of the H->H convs is folded into the center tap (W += I), and
    conv biases are dropped (they cancel inside BatchNorm). Conv1's 271
    input channels x 3 taps pack into 7 matmul passes.
  - Per layer per stream: convs (PE) -> evict PSUM->SBUF (split DVE/ACT) ->
    bn_stats in 512-element chunks (DVE) -> partial-sum decode (DVE) ->
    4-group AllGather -> scale/shift decode (Pool + one ACT sqrt; no DVE op
    depends on the collective, so DVE never stalls) -> batched gelu apply
    (ACT).
  - Head: time-mean pooling via DVE reduces; pooled @ headW[:, :128] in bf16
    on PE; the per-subject constant (headW[:,128:] @ emb[s] + headb) is added
    during host-side unsharding.
"""

import numpy as np

_CACHE = {}
_SECTIONS = []

N_CORES = 8
CIN = 271
T = 281
TP = 284  # padded time: col 0 zero, cols 1..281 data, cols 282..283 zero
H = 128
S = 4
NCLS = 1854
NCHUNKS = (NCLS + 127) // 128  # 15
EPS = 1e-5
GS = 8  # samples per slot group
_EVOFF = 60  # evict priority boost (instructions)
CHUNK_SIZES = [512, 512, 512, 512, 224]  # per-group bn_stats chunking of 8*284


def _build(KA, KB):
    import concourse.bacc as bacc
    import concourse.tile as tile
    import concourse.mybir as mybir

    f32 = mybir.dt.float32
    f32r = mybir.dt.float32r
    bf16 = mybir.dt.bfloat16
    AF = mybir.ActivationFunctionType
    OP = mybir.AluOpType
    AX = mybir.AxisListType

    K = KA + KB
    WA = KA // GS
    WB = KB // GS
    W = WA + WB
    NCH_A = 5 * WA
    NCH_B = 5 * WB
    NCH = NCH_A + NCH_B
    assert sum(CHUNK_SIZES) == GS * TP

    # block descriptors: (slab range, chunk offset, ref index)
    BLK = [
        dict(w0=0, w1=WA, c0=0, c1=NCH_A, ref=NCH, nch=NCH_A),
        dict(w0=WA, w1=W, c0=NCH_A, c1=NCH, ref=NCH + 1, nch=NCH_B),
    ]

    nc = bacc.Bacc("TRN2", target_bir_lowering=False, num_devices=N_CORES)

    # ---- DRAM I/O ----
    Xd = nc.dram_tensor("Xd", [K, 301, TP], bf16, kind="ExternalInput")
    W1d = nc.dram_tensor("W1d", [128, 7 * 128], bf16, kind="ExternalInput")
    WRd = nc.dram_tensor("WRd", [128, 24 * 128], f32r, kind="ExternalInput")
    GAMd = nc.dram_tensor("GAMd", [128, 18], f32, kind="ExternalInput")
    BETd = nc.dram_tensor("BETd", [128, 18], f32, kind="ExternalInput")
    INVCd = nc.dram_tensor("INVCd", [128, 2], f32, kind="ExternalInput")
    NPADd = nc.dram_tensor("NPADd", [128, 2], f32, kind="ExternalInput")
    CECd = nc.dram_tensor("CECd", [128, NCH], f32, kind="ExternalInput")
    EMBHd = nc.dram_tensor("EMBHd", [128, NCHUNKS * 128], bf16, kind="ExternalInput")
    OUTd = nc.dram_tensor("OUTd", [NCLS, K], f32, kind="ExternalOutput")

    cc_in = [[nc.dram_tensor(f"ccin{l}_{b}", [128, 2], f32) for b in range(2)]
             for l in range(9)]
    cc_out = [[nc.dram_tensor(f"ccout{l}_{b}", [4, 128, 2], f32) for b in range(2)]
              for l in range(9)]
    groups4 = [[0, 1, 2, 3], [4, 5, 6, 7]]

    with tile.TileContext(nc) as tc:
        # ---- static SBUF ----
        W1s = nc.alloc_sbuf_tensor("W1s", [128, 7 * 128], bf16)
        WRs = nc.alloc_sbuf_tensor("WRs", [128, 24 * 128], f32r)
        GAMs = nc.alloc_sbuf_tensor("GAMs", [128, 18], f32)
        BETs = nc.alloc_sbuf_tensor("BETs", [128, 18], f32)
        INVCs = nc.alloc_sbuf_tensor("INVCs", [128, 2], f32)
        NPADs = nc.alloc_sbuf_tensor("NPADs", [128, 2], f32)
        CECs = nc.alloc_sbuf_tensor("CECs", [128, NCH], f32)
        EMBHs = nc.alloc_sbuf_tensor("EMBHs", [128, NCHUNKS * 128], bf16)
        BNSTs = nc.alloc_sbuf_tensor("BNSTs", [128, (NCH + 2) * 6], f32)
        dA = nc.alloc_sbuf_tensor("dA", [128, max(NCH_A, NCH_B)], f32)
        dB = nc.alloc_sbuf_tensor("dB", [128, max(NCH_A, NCH_B)], f32)
        dC = nc.alloc_sbuf_tensor("dC", [128, max(NCH_A, NCH_B)], f32)
        SST = nc.alloc_sbuf_tensor("SST", [128, 4], f32)   # A: 0:2, B: 2:4
        SG2 = [nc.alloc_sbuf_tensor(f"SG2_{b}", [128, 8], f32) for b in range(2)]
        SG = [nc.alloc_sbuf_tensor(f"SG_{b}", [128, 2], f32) for b in range(2)]
        sm = [[nc.alloc_sbuf_tensor(f"sm{b}_{i}", [128, 1], f32) for i in range(10)]
              for b in range(2)]
        EPSs = nc.alloc_sbuf_tensor("EPSs", [128, 1], f32)
        HALFTPs = nc.alloc_sbuf_tensor("HALFTPs", [128, 1], f32)
        MAGICs = nc.alloc_sbuf_tensor("MAGICs", [128, 1], f32)  # rsqrt seed bits
        SHIFT1s = nc.alloc_sbuf_tensor("SHIFT1s", [128, 1], f32)  # int 1 (shift amt)
        NHALFs = nc.alloc_sbuf_tensor("NHALFs", [128, 1], f32)
        THALFs = nc.alloc_sbuf_tensor("THALFs", [128, 1], f32)
        ZBIG = nc.alloc_sbuf_tensor("ZBIG", [128, TP], f32)
        P0 = nc.alloc_sbuf_tensor("P0", [128, K], f32)
        P0h = nc.alloc_sbuf_tensor("P0h", [128, K], bf16)
        ysg = [nc.alloc_sbuf_tensor(f"ysg{w}", [128, GS * TP], f32r)
               for w in range(W)]
        ysr = [nc.alloc_sbuf_tensor(f"ysr{b}", [128, TP], f32r) for b in range(2)]

        with tc.tile_pool(name="xpool", bufs=4) as xpool, \
             tc.tile_pool(name="xpool2", bufs=2) as xpool2, \
             tc.tile_pool(name="hcpool", bufs=8) as hcpool:

            # constant loads
            nc.sync.dma_start(out=W1s.ap(), in_=W1d.ap())
            nc.sync.dma_start(out=WRs.ap(), in_=WRd.ap())
            nc.sync.dma_start(out=GAMs.ap(), in_=GAMd.ap())
            nc.sync.dma_start(out=BETs.ap(), in_=BETd.ap())
            nc.sync.dma_start(out=INVCs.ap(), in_=INVCd.ap())
            nc.sync.dma_start(out=NPADs.ap(), in_=NPADd.ap())
            nc.sync.dma_start(out=CECs.ap(), in_=CECd.ap())
            nc.sync.dma_start(out=EMBHs.ap(), in_=EMBHd.ap())
            nc.gpsimd.memset(EPSs.ap(), EPS)
            nc.gpsimd.memset(HALFTPs.ap(), float(TP // 2))
            # float whose bit pattern is 0x5f3759df (fast-rsqrt magic)
            nc.gpsimd.memset(MAGICs.ap(), 13211836172961054720.0)
            # float whose bit pattern is int 1 (shift amount as int32)
            nc.gpsimd.memset(SHIFT1s.ap(), 1.401298464324817e-45)
            nc.gpsimd.memset(NHALFs.ap(), -0.5)
            nc.gpsimd.memset(THALFs.ap(), 1.5)
            nc.gpsimd.memset(ZBIG.ap(), 0.0)
            # zero the pad columns {0, 282, 283} of every slot
            for w in range(W):
                y3 = ysg[w].ap().rearrange("p (j t) -> p j t", j=GS)
                nc.gpsimd.tensor_copy(
                    out=y3[:, :, 0:1],
                    in_=ZBIG.ap()[:, 0:GS].rearrange("p (j o) -> p j o", o=1))
                nc.gpsimd.tensor_copy(
                    out=y3[:, :, 282:284],
                    in_=ZBIG.ap()[:, 0:2 * GS].rearrange("p (j o) -> p j o", o=2))
            for b in range(2):
                nc.gpsimd.tensor_copy(out=ysr[b].ap()[:, 0:1], in_=ZBIG.ap()[:, 0:1])
                nc.gpsimd.tensor_copy(out=ysr[b].ap()[:, 282:284], in_=ZBIG.ap()[:, 0:2])

            def evict_engine(l, w, half):
                # Returns 'v' (DVE) or 'a' (ACT). ACT takes the EARLY slabs
                # of each stream section (so its evicts finish before the
                # interleaved apply of the other stream needs ACT); DVE takes
                # the LATE slabs. Totals per layer: ~10-12 DVE, ~22-24 ACT.
                blk = 0 if w < WA else 1
                if l == 0:
                    return 'v' if (w % 2 == 0) else 'a'
                if blk == 0:
                    return 'a' if w < 4 else 'v'
                else:
                    return 'a' if (w - WA) < 7 else 'v'

            def evict_half(l, w, half, ps):
                y3 = ysg[w].ap().rearrange("p (j t) -> p j t", j=GS)
                src = ps[:].rearrange("p (j t) -> p j t", j=4)[:, :, 0:T]
                dst = y3[:, 4 * half:4 * half + 4, 1:1 + T]
                with tc.high_priority(offset=_EVOFF):
                    if evict_engine(l, w, half) == 'v':
                        nc.vector.tensor_copy(out=dst, in_=src)
                    else:
                        nc.scalar.activation(out=dst, in_=src, func=AF.Copy)

            def slab_stats(w):
                off = 0
                for i, csz in enumerate(CHUNK_SIZES):
                    c = 5 * w + i
                    nc.vector.bn_stats(out=BNSTs.ap()[:, 6 * c:6 * c + 6],
                                       in_=ysg[w].ap()[:, off:off + csz])
                    off += csz

            def ref_stats(b):
                r = BLK[b]['ref']
                nc.vector.bn_stats(out=BNSTs.ap()[:, 6 * r:6 * r + 6],
                                   in_=ysr[b].ap())

            def pre_decode(l, b):
                # partial-sum fold (DVE) -> SST[:, 2b:2b+2] -> dram ccin
                blk = BLK[b]
                c0, nch = blk['c0'], blk['nch']
                bn3 = BNSTs.ap().rearrange("p (c s) -> p c s", s=6)
                ME = bn3[:, c0:c0 + nch, 1]
                MO = bn3[:, c0:c0 + nch, 4]
                CVE = bn3[:, c0:c0 + nch, 2]
                CVO = bn3[:, c0:c0 + nch, 5]
                CEC = CECs.ap()[:, c0:c0 + nch]
                a = dA.ap()[:, 0:nch]
                bb = dB.ap()[:, 0:nch]
                cc = dC.ap()[:, 0:nch]
                nc.vector.tensor_tensor(out=a, in0=ME, in1=MO, op=OP.add)
                nc.vector.tensor_tensor(out=a, in0=a, in1=CEC, op=OP.mult)
                nc.vector.tensor_reduce(out=SST.ap()[:, 2 * b:2 * b + 1], in_=a,
                                        axis=AX.X, op=OP.add)
                nc.vector.tensor_tensor(out=bb, in0=ME, in1=ME, op=OP.mult)
                nc.vector.tensor_tensor(out=cc, in0=MO, in1=MO, op=OP.mult)
                nc.vector.tensor_tensor(out=bb, in0=bb, in1=cc, op=OP.add)
                nc.vector.tensor_tensor(out=bb, in0=bb, in1=CEC, op=OP.mult)
                nc.vector.tensor_tensor(out=bb, in0=bb, in1=CVE, op=OP.add)
                nc.vector.tensor_tensor(out=bb, in0=bb, in1=CVO, op=OP.add)
                nc.vector.tensor_reduce(out=SST.ap()[:, 2 * b + 1:2 * b + 2], in_=bb,
                                        axis=AX.X, op=OP.add)
                nc.sync.dma_start(out=cc_in[l][b].ap(), in_=SST.ap()[:, 2 * b:2 * b + 2])

            def launch_cc(l, b):
                nc.gpsimd.collective_compute(
                    "AllGather", OP.bypass, replica_groups=groups4,
                    ins=[cc_in[l][b].ap()], outs=[cc_out[l][b].ap()])

            def refpad_decode(b):
                # ref-pad contribution, computed on Pool while the CC flies
                rb = 6 * BLK[b]['ref']
                MEr = BNSTs.ap()[:, rb + 1:rb + 2]
                MOr = BNSTs.ap()[:, rb + 4:rb + 5]
                CVEr = BNSTs.ap()[:, rb + 2:rb + 3]
                CVOr = BNSTs.ap()[:, rb + 5:rb + 6]
                s1r, s2r, t0 = sm[b][0], sm[b][1], sm[b][2]
                g = nc.gpsimd
                g.tensor_tensor(out=s1r.ap(), in0=MEr, in1=MOr, op=OP.add)
                g.tensor_tensor(out=s1r.ap(), in0=s1r.ap(), in1=HALFTPs.ap(),
                                op=OP.mult)
                g.tensor_tensor(out=s2r.ap(), in0=MEr, in1=MEr, op=OP.mult)
                g.tensor_tensor(out=t0.ap(), in0=MOr, in1=MOr, op=OP.mult)
                g.tensor_tensor(out=s2r.ap(), in0=s2r.ap(), in1=t0.ap(), op=OP.add)
                g.tensor_tensor(out=s2r.ap(), in0=s2r.ap(), in1=HALFTPs.ap(),
                                op=OP.mult)
                g.tensor_tensor(out=s2r.ap(), in0=s2r.ap(), in1=CVEr, op=OP.add)
                g.tensor_tensor(out=s2r.ap(), in0=s2r.ap(), in1=CVOr, op=OP.add)
                g.tensor_tensor(out=s1r.ap(), in0=s1r.ap(),
                                in1=NPADs.ap()[:, b:b + 1], op=OP.mult)
                g.tensor_tensor(out=s2r.ap(), in0=s2r.ap(),
                                in1=NPADs.ap()[:, b:b + 1], op=OP.mult)

            def post_decode(l, b):
                # gather result -> scale/shift on Pool (idle engine).
                g = nc.gpsimd
                nc.scalar.dma_start(
                    out=SG2[b].ap().rearrange("p (g s) -> p g s", g=4),
                    in_=cc_out[l][b].ap().rearrange("g p s -> p g s"))
                sg2 = SG2[b].ap().rearrange("p (g s) -> p g s", g=4)
                sg = SG[b].ap()
                g.tensor_tensor(out=sg, in0=sg2[:, 0, :], in1=sg2[:, 1, :], op=OP.add)
                g.tensor_tensor(out=sg, in0=sg, in1=sg2[:, 2, :], op=OP.add)
                g.tensor_tensor(out=sg, in0=sg, in1=sg2[:, 3, :], op=OP.add)
                s1r, s2r = sm[b][0], sm[b][1]
                g.tensor_tensor(out=sg[:, 0:1], in0=sg[:, 0:1], in1=s1r.ap(),
                                op=OP.subtract)
                g.tensor_tensor(out=sg[:, 1:2], in0=sg[:, 1:2], in1=s2r.ap(),
                                op=OP.subtract)
                meanv, msqv, varv, sdv, invv, sclv, sftv = (
                    sm[b][3], sm[b][4], sm[b][5], sm[b][6], sm[b][7], sm[b][8],
                    sm[b][9])
                invc = INVCs.ap()[:, b:b + 1]
                g.tensor_tensor(out=meanv.ap(), in0=sg[:, 0:1], in1=invc, op=OP.mult)
                g.tensor_tensor(out=msqv.ap(), in0=sg[:, 1:2], in1=invc, op=OP.mult)
                g.tensor_tensor(out=varv.ap(), in0=meanv.ap(), in1=meanv.ap(),
                                op=OP.mult)
                g.tensor_tensor(out=varv.ap(), in0=msqv.ap(), in1=varv.ap(),
                                op=OP.subtract)
                # inv = rsqrt(var + eps) entirely on Pool (bit-trick seed +
                # 2 Newton steps): ACT never leaves the gelu table set, and
                # DVE has no collective-dependent instruction anywhere.
                i32t = mybir.dt.int32
                v = sdv  # reuse scratch: v = var + eps
                g.tensor_tensor(out=v.ap(), in0=varv.ap(), in1=EPSs.ap(), op=OP.add)
                t0 = sm[b][2]
                nc.vector.tensor_tensor(
                    out=t0.ap().bitcast(i32t), in0=v.ap().bitcast(i32t),
                    in1=SHIFT1s.ap().bitcast(i32t), op=OP.arith_shift_right)
                nc.vector.tensor_tensor(
                    out=invv.ap().bitcast(i32t), in0=MAGICs.ap().bitcast(i32t),
                    in1=t0.ap().bitcast(i32t), op=OP.subtract)
                for _ in range(2):
                    g.tensor_tensor(out=t0.ap(), in0=invv.ap(),
                                    in1=invv.ap(), op=OP.mult)
                    g.tensor_tensor(out=t0.ap(), in0=t0.ap(),
                                    in1=v.ap(), op=OP.mult)
                    g.tensor_tensor(out=t0.ap(), in0=t0.ap(),
                                    in1=NHALFs.ap(), op=OP.mult)
                    g.tensor_tensor(out=t0.ap(), in0=t0.ap(),
                                    in1=THALFs.ap(), op=OP.add)
                    g.tensor_tensor(out=invv.ap(), in0=invv.ap(),
                                    in1=t0.ap(), op=OP.mult)
                gam = GAMs.ap()[:, 2 * l + b:2 * l + b + 1]
                bet = BETs.ap()[:, 2 * l + b:2 * l + b + 1]
                g.tensor_tensor(out=sclv.ap(), in0=gam, in1=invv.ap(), op=OP.mult)
                g.tensor_tensor(out=sftv.ap(), in0=sclv.ap(), in1=meanv.ap(),
                                op=OP.mult)
                g.tensor_tensor(out=sftv.ap(), in0=bet, in1=sftv.ap(), op=OP.subtract)
                return sclv, sftv

            def apply_slab(b, w, sclv, sftv, pool=False):
                y3 = ysg[w].ap().rearrange("p (j t) -> p j t", j=GS)
                nc.scalar.activation(out=y3[:, :, 1:1 + T], in_=y3[:, :, 1:1 + T],
                                     func=AF.Gelu, bias=sftv.ap(),
                                     scale=sclv.ap())
                if pool:
                    p03 = P0.ap().rearrange("p (k o) -> p k o", o=1)
                    with tc.high_priority():
                        nc.vector.tensor_reduce(
                            out=p03[:, GS * w:GS * w + GS, :],
                            in_=y3[:, :, 1:1 + T], axis=AX.X, op=OP.add)

            def apply_ref(b, sclv, sftv):
                nc.scalar.activation(out=ysr[b].ap()[:, 1:1 + T],
                                     in_=ysr[b].ap()[:, 1:1 + T],
                                     func=AF.Gelu, bias=sftv.ap(), scale=sclv.ap())

            # ================= layers =================
            with tc.tile_pool(name="pspool", bufs=2, space="PSUM") as pspool:
                def section(l, b, pspool):
                    # Fused section for (layer l, stream b): decode the
                    # previous layer's collective (Pool), then per slab:
                    # apply(l-1) [ACT] -> conv(l) [PE] -> evict [ACT/DVE] ->
                    # bn_stats [DVE]; finally fold partial sums and launch
                    # this layer's collective.
                    _SECTIONS.append((f"conv{l}_{'AB'[b]}", nc.next_id()))
                    blk = BLK[b]
                    if l > 0:
                        sclv, sftv = post_decode(l - 1, b)
                    if True:
                        if l == 0:
                            for w in range(blk['w0'], blk['w1']):
                                pss = []
                                for half in range(2):
                                    ps = pspool.tile([128, 2048], f32, tag="ps")
                                    pss.append(ps)
                                    for j2 in range(2):
                                        smp = GS * w + 4 * half + 2 * j2
                                        x0 = xpool.tile([128, 2, TP], bf16, tag="xc0")
                                        x1 = xpool.tile([128, 2, TP], bf16, tag="xc1")
                                        x2 = xpool2.tile([45, 2, TP], bf16, tag="xc2")
                                        q = nc.sync if (j2 == 0) else nc.scalar
                                        q.dma_start(out=x0[:], in_=Xd.ap()[smp:smp + 2, 0:128, :].rearrange("b c t -> c b t"))
                                        q.dma_start(out=x1[:], in_=Xd.ap()[smp:smp + 2, 128:256, :].rearrange("b c t -> c b t"))
                                        q.dma_start(out=x2[:], in_=Xd.ap()[smp:smp + 2, 256:301, :].rearrange("b c t -> c b t"))
                                        for jj in range(2):
                                            o = 512 * (2 * j2 + jj)
                                            idx = 0
                                            for xt, base in ((x0, 0), (x1, 3)):
                                                for k in range(3):
                                                    nc.tensor.matmul(
                                                        ps[:, o:o + 282],
                                                        W1s.ap()[:, (base + k) * 128:(base + k + 1) * 128],
                                                        xt[:, jj, k:k + 282],
                                                        start=(idx == 0), stop=False)
                                                    idx += 1
                                            nc.tensor.matmul(
                                                ps[:, o:o + 282],
                                                W1s.ap()[0:45, 6 * 128:7 * 128],
                                                x2[0:45, jj, 1:283], start=False, stop=True)
                                    evict_half(l, w, half, ps)
                                slab_stats(w)
                            # ref slot: conv(0) == 0
                            nc.scalar.activation(out=ysr[b].ap()[:, 1:1 + T],
                                                 in_=ZBIG.ap()[:, 0:T], func=AF.Copy)
                            ref_stats(b)
                        else:
                            w0 = (l - 1) * 3
                            apply_slab(b, blk['w0'], sclv, sftv)
                            for w in range(blk['w0'], blk['w1']):
                                pss = []
                                for half in range(2):
                                    ps = pspool.tile([128, 2048], f32, tag="ps")
                                    pss.append(ps)
                                    for j4 in range(4):
                                        j = 4 * half + j4
                                        o = 512 * j4
                                        for k in range(3):
                                            nc.tensor.matmul(
                                                ps[:, o:o + 282],
                                                WRs.ap()[:, (w0 + k) * 128:(w0 + k + 1) * 128],
                                                ysg[w].ap()[:, j * TP + k:j * TP + k + 282],
                                                start=(k == 0), stop=(k == 2))
                                    evict_half(l, w, half, ps)
                                    if half == 0 and w + 1 < blk['w1']:
                                        apply_slab(b, w + 1, sclv, sftv)
                                slab_stats(w)
                            # ref slot conv
                            apply_ref(b, sclv, sftv)
                            psr = pspool.tile([128, 2048], f32, tag="ps")
                            for k in range(3):
                                nc.tensor.matmul(
                                    psr[:, 0:282],
                                    WRs.ap()[:, (w0 + k) * 128:(w0 + k + 1) * 128],
                                    ysr[b].ap()[:, k:k + 282],
                                    start=(k == 0), stop=(k == 2))
                            nc.scalar.activation(out=ysr[b].ap()[:, 1:1 + T],
                                                 in_=psr[:, 0:T], func=AF.Copy)
                            ref_stats(b)
                        pre_decode(l, b)
                        launch_cc(l, b)
                        refpad_decode(b)

                def final_apply(b):
                    _SECTIONS.append((f"apply8_{'AB'[b]}", nc.next_id()))
                    blk = BLK[b]
                    sclv, sftv = post_decode(8, b)
                    for w in range(blk['w0'], blk['w1']):
                        apply_slab(b, w, sclv, sftv, pool=True)
                    apply_ref(b, sclv, sftv)

                for l in range(9):
                    section(l, 0, pspool)
                    section(l, 1, pspool)
                final_apply(0)
                final_apply(1)
                nc.vector.tensor_copy(out=P0h.ap(), in_=P0.ap())

            _SECTIONS.append(("head", nc.next_id()))
            # ================= head =================
            with tc.tile_pool(name="hppool", bufs=8, space="PSUM") as hppool:
                for n in range(NCHUNKS):
                    rows = min(128, NCLS - n * 128)
                    hp = hppool.tile([128, K], f32, tag="hp")
                    nc.tensor.matmul(hp[:], EMBHs.ap()[:, n * 128:(n + 1) * 128],
                                     P0h.ap(), start=True, stop=True)
                    hc = hcpool.tile([128, K], f32, tag="hc")
                    nc.scalar.activation(out=hc[:], in_=hp[:], func=AF.Copy)
                    nc.sync.dma_start(out=OUTd.ap()[n * 128:n * 128 + rows, :],
                                      in_=hc[0:rows, :])

    nc.finalize()
    return nc


def kernel(**inputs):
    from concourse.bass_utils import run_bass_kernel_spmd
    import ml_dtypes

    X = np.asarray(inputs["X"], dtype=np.float32)
    w1_0 = np.asarray(inputs["w1_0"], dtype=np.float32)
    w_rest = np.asarray(inputs["w_rest"], dtype=np.float32)
    gammas = np.asarray(inputs["gammas"], dtype=np.float32)
    betas = np.asarray(inputs["betas"], dtype=np.float32)
    emb = np.asarray(inputs["emb"], dtype=np.float32)
    headW = np.asarray(inputs["headW"], dtype=np.float32)
    headb = np.asarray(inputs["headb"], dtype=np.float32)
    sidx = np.asarray(inputs["subject_idxs"]).astype(np.int64)

    B = X.shape[0]
    counts = np.bincount(sidx, minlength=S)
    order = np.argsort(sidx, kind="stable")
    offs = np.zeros(S + 1, np.int64)
    offs[1:] = np.cumsum(counts)

    def r8(x):
        return ((int(x) + GS - 1) // GS) * GS

    # A-stream: two largest subjects; B-stream: two smallest.
    desc = list(np.argsort(-counts, kind="stable"))
    subjA = [int(desc[0]), int(desc[1])]   # group 0, group 1
    subjB = [int(desc[2]), int(desc[3])]
    KA = r8(max(GS, -(-counts[subjA[0]] // 4), -(-counts[subjA[1]] // 4)))
    KB = r8(max(GS, -(-counts[subjB[0]] // 4), -(-counts[subjB[1]] // 4)))
    K = KA + KB

    quarters = {}
    for s in range(S):
        ids = order[offs[s]:offs[s + 1]]
        quarters[s] = np.array_split(ids, 4)

    # ---- shared host-side weight prep ----
    W1p = np.zeros((128, 7 * 128), np.float32)
    for c in range(2):
        for k in range(3):
            W1p[:, (c * 3 + k) * 128:(c * 3 + k + 1) * 128] = \
                w1_0[:, c * 128:(c + 1) * 128, k].T
    for k in range(3):
        W1p[15 * k:15 * k + 15, 6 * 128:7 * 128] = w1_0[:, 256:271, k].T
    WRp = np.zeros((128, 24 * 128), np.float32)
    eye = np.eye(H, dtype=np.float32)
    for l in range(8):
        for k in range(3):
            wt = w_rest[l, :, :, k].T.copy()
            if k == 1:
                wt += eye
            WRp[:, (l * 3 + k) * 128:(l * 3 + k + 1) * 128] = wt
    EMBHp = np.zeros((128, NCHUNKS * 128), np.float32)
    EMBHp[:, 0:NCLS] = headW[:, 0:H].T / float(T)
    WAn = KA // GS
    WBn = KB // GS
    CECp = np.tile(np.array([[c // 2 for c in CHUNK_SIZES]], np.float32),
                   (128, WAn + WBn)).astype(np.float32)

    in_maps = []
    core_ids_list = []
    for c in range(N_CORES):
        g = c // 4
        q = c % 4
        sA, sB = subjA[g], subjB[g]
        idsA = quarters[sA][q]
        idsB = quarters[sB][q]
        core_ids_list.append((idsA, idsB))
        Xc = np.zeros((K, 301, TP), np.float32)
        for ids, base in ((idsA, 0), (idsB, KA)):
            n = len(ids)
            if n:
                Xc[base:base + n, 0:CIN, 1:1 + T] = X[ids]
                # pre-shifted 15-channel tail bands
                xt = Xc[base:base + n, 256:CIN, :].copy()
                Xc[base:base + n, 256:271, 1:] = xt[:, :, :-1]
                Xc[base:base + n, 256:271, 0] = 0.0
                Xc[base:base + n, 271:286, :] = xt
                Xc[base:base + n, 286:301, :-1] = xt[:, :, 1:]
                Xc[base:base + n, 286:301, -1] = 0.0
        INVC = np.zeros((128, 2), np.float32)
        INVC[:, 0] = 1.0 / (max(int(counts[sA]), 1) * T)
        INVC[:, 1] = 1.0 / (max(int(counts[sB]), 1) * T)
        NPAD = np.zeros((128, 2), np.float32)
        NPAD[:, 0] = float(4 * KA - int(counts[sA]))
        NPAD[:, 1] = float(4 * KB - int(counts[sB]))
        GAMp = np.zeros((128, 18), np.float32)
        BETp = np.zeros((128, 18), np.float32)
        for l in range(9):
            GAMp[:, 2 * l + 0] = gammas[l, sA, :]
            GAMp[:, 2 * l + 1] = gammas[l, sB, :]
            BETp[:, 2 * l + 0] = betas[l, sA, :]
            BETp[:, 2 * l + 1] = betas[l, sB, :]
        in_maps.append({
            "Xd": Xc.astype(ml_dtypes.bfloat16),
            "W1d": W1p.astype(ml_dtypes.bfloat16),
            "WRd": WRp,
            "GAMd": GAMp,
            "BETd": BETp,
            "INVCd": INVC,
            "NPADd": NPAD,
            "CECd": CECp,
            "EMBHd": EMBHp.astype(ml_dtypes.bfloat16),
        })

    key = (KA, KB)
    if key not in _CACHE:
        _CACHE[key] = _build(KA, KB)
    nc = _CACHE[key]

    res = run_bass_kernel_spmd(nc, in_maps, core_ids=list(range(N_CORES)))
    kernel.last_results = res

    out = np.zeros((B, NCLS), np.float32)
    b2 = emb @ headW[:, H:].T + headb[None, :]  # [S, NCLS]
    for c in range(N_CORES):
        g = c // 4
        idsA, idsB = core_ids_list[c]
        resc = res.results[c]["OUTd"].T  # [K, NCLS]
        if len(idsA):
            out[idsA] = resc[0:len(idsA)] + b2[subjA[g]][None, :]
        if len(idsB):
            out[idsB] = resc[KA:KA + len(idsB)] + b2[subjB[g]][None, :]
    return out


# revision 22
# speedup vs baseline: 1.2076x; 1.2076x over previous
"""BasicConvClassifier on 8 Trainium2 NeuronCores.

Strategy (quarter-subject sharding with A/B stream interleave):
  - Sort the batch by subject (4 subjects). The two largest subjects become
    "A-stream" subjects, the two smallest "B-stream". Cores 0-3 share the
    quarters of A-subject#0 and B-subject#0; cores 4-7 share A-subject#1 /
    B-subject#1. Each core holds KA A-samples and KB B-samples (padded with
    zero samples to uniform KA/KB, multiple of 8).
  - Per-subject BN stats reduce over a 4-core AllGather group. Each layer
    issues TWO collectives (A then B); the 15us collective latency of stream
    A is hidden under stream B's conv/stats work and vice versa, so the
    per-layer sync cost mostly disappears from the critical path.
  - Pad samples follow a dedicated reference-pad slot (X = 0) per stream;
    their stats contribution is subtracted exactly as npad * ref.
  - Conv1d(k=3, SAME) is shifted fp32r matmuls accumulated in PSUM; the
    residual of the H->H convs is folded into the center tap (W += I), and
    conv biases are dropped (they cancel inside BatchNorm). Conv1's 271
    input channels x 3 taps pack into 7 matmul passes.
  - Per layer per stream: convs (PE) -> evict PSUM->SBUF (split DVE/ACT) ->
    bn_stats in 512-element chunks (DVE) -> partial-sum decode (DVE) ->
    4-group AllGather -> scale/shift decode (Pool + one ACT sqrt; no DVE op
    depends on the collective, so DVE never stalls) -> batched gelu apply
    (ACT).
  - Head: time-mean pooling via DVE reduces; pooled @ headW[:, :128] in bf16
    on PE; the per-subject constant (headW[:,128:] @ emb[s] + headb) is added
    during host-side unsharding.
"""

import numpy as np

_CACHE = {}
_SECTIONS = []

N_CORES = 8
CIN = 271
T = 281
TP = 284  # padded time: col 0 zero, cols 1..281 data, cols 282..283 zero
H = 128
S = 4
NCLS = 1854
NCHUNKS = (NCLS + 127) // 128  # 15
EPS = 1e-5
GS = 8  # samples per slot group
_EVOFF = 60  # evict priority boost (instructions)
CHUNK_SIZES = [512, 512, 512, 512, 224]  # per-group bn_stats chunking of 8*284


def _build(KA, KB):
    import concourse.bacc as bacc
    import concourse.tile as tile
    import concourse.mybir as mybir

    f32 = mybir.dt.float32
    f32r = mybir.dt.float32r
    bf16 = mybir.dt.bfloat16
    AF = mybir.ActivationFunctionType
    OP = mybir.AluOpType
    AX = mybir.AxisListType

    K = KA + KB
    WA = KA // GS
    WB = KB // GS
    W = WA + WB
    NCH_A = 5 * WA
    NCH_B = 5 * WB
    NCH = NCH_A + NCH_B
    assert sum(CHUNK_SIZES) == GS * TP

    # block descriptors: (slab range, chunk offset, ref index)
    BLK = [
        dict(w0=0, w1=WA, c0=0, c1=NCH_A, ref=NCH, nch=NCH_A),
        dict(w0=WA, w1=W, c0=NCH_A, c1=NCH, ref=NCH + 1, nch=NCH_B),
    ]

    nc = bacc.Bacc("TRN2", target_bir_lowering=False, num_devices=N_CORES)

    # ---- DRAM I/O ----
    Xd = nc.dram_tensor("Xd", [K, 301, TP], bf16, kind="ExternalInput")
    W1d = nc.dram_tensor("W1d", [128, 7 * 128], bf16, kind="ExternalInput")
    WRd = nc.dram_tensor("WRd", [128, 24 * 128], f32r, kind="ExternalInput")
    GAMd = nc.dram_tensor("GAMd", [128, 18], f32, kind="ExternalInput")
    BETd = nc.dram_tensor("BETd", [128, 18], f32, kind="ExternalInput")
    INVCd = nc.dram_tensor("INVCd", [128, 2], f32, kind="ExternalInput")
    NPADd = nc.dram_tensor("NPADd", [128, 2], f32, kind="ExternalInput")
    CECd = nc.dram_tensor("CECd", [128, NCH], f32, kind="ExternalInput")
    EMBHd = nc.dram_tensor("EMBHd", [128, NCHUNKS * 128], bf16, kind="ExternalInput")
    OUTd = nc.dram_tensor("OUTd", [NCLS, K], f32, kind="ExternalOutput")

    cc_in = [[nc.dram_tensor(f"ccin{l}_{b}", [128, 2], f32) for b in range(2)]
             for l in range(9)]
    cc_out = [[nc.dram_tensor(f"ccout{l}_{b}", [4, 128, 2], f32) for b in range(2)]
              for l in range(9)]
    groups4 = [[0, 1, 2, 3], [4, 5, 6, 7]]

    with tile.TileContext(nc) as tc:
        # ---- static SBUF ----
        W1s = nc.alloc_sbuf_tensor("W1s", [128, 7 * 128], bf16)
        WRs = nc.alloc_sbuf_tensor("WRs", [128, 24 * 128], f32r)
        GAMs = nc.alloc_sbuf_tensor("GAMs", [128, 18], f32)
        BETs = nc.alloc_sbuf_tensor("BETs", [128, 18], f32)
        INVCs = nc.alloc_sbuf_tensor("INVCs", [128, 2], f32)
        NPADs = nc.alloc_sbuf_tensor("NPADs", [128, 2], f32)
        CECs = nc.alloc_sbuf_tensor("CECs", [128, NCH], f32)
        EMBHs = nc.alloc_sbuf_tensor("EMBHs", [128, NCHUNKS * 128], bf16)
        BNSTs = nc.alloc_sbuf_tensor("BNSTs", [128, (NCH + 2) * 6], f32)
        dA = nc.alloc_sbuf_tensor("dA", [128, max(NCH_A, NCH_B)], f32)
        dB = nc.alloc_sbuf_tensor("dB", [128, max(NCH_A, NCH_B)], f32)
        dC = nc.alloc_sbuf_tensor("dC", [128, max(NCH_A, NCH_B)], f32)
        SST = nc.alloc_sbuf_tensor("SST", [128, 4], f32)   # A: 0:2, B: 2:4
        SG2 = [nc.alloc_sbuf_tensor(f"SG2_{b}", [128, 8], f32) for b in range(2)]
        SG = [nc.alloc_sbuf_tensor(f"SG_{b}", [128, 2], f32) for b in range(2)]
        sm = [[nc.alloc_sbuf_tensor(f"sm{b}_{i}", [128, 1], f32) for i in range(10)]
              for b in range(2)]
        EPSs = nc.alloc_sbuf_tensor("EPSs", [128, 1], f32)
        HALFTPs = nc.alloc_sbuf_tensor("HALFTPs", [128, 1], f32)
        MAGICs = nc.alloc_sbuf_tensor("MAGICs", [128, 1], f32)  # rsqrt seed bits
        SHIFT1s = nc.alloc_sbuf_tensor("SHIFT1s", [128, 1], f32)  # int 1 (shift amt)
        INVCLs = nc.alloc_sbuf_tensor("INVCLs", [128, 2], f32)  # local 1/(K_blk*T)
        NHALFs = nc.alloc_sbuf_tensor("NHALFs", [128, 1], f32)
        THALFs = nc.alloc_sbuf_tensor("THALFs", [128, 1], f32)
        ZBIG = nc.alloc_sbuf_tensor("ZBIG", [128, TP], f32)
        P0 = nc.alloc_sbuf_tensor("P0", [128, K], f32)
        P0h = nc.alloc_sbuf_tensor("P0h", [128, K], bf16)
        ysg = [nc.alloc_sbuf_tensor(f"ysg{w}", [128, GS * TP], f32r)
               for w in range(W)]
        ysr = [nc.alloc_sbuf_tensor(f"ysr{b}", [128, TP], f32r) for b in range(2)]

        with tc.tile_pool(name="xpool", bufs=4) as xpool, \
             tc.tile_pool(name="xpool2", bufs=2) as xpool2, \
             tc.tile_pool(name="hcpool", bufs=8) as hcpool:

            # constant loads
            nc.sync.dma_start(out=W1s.ap(), in_=W1d.ap())
            nc.sync.dma_start(out=WRs.ap(), in_=WRd.ap())
            nc.sync.dma_start(out=GAMs.ap(), in_=GAMd.ap())
            nc.sync.dma_start(out=BETs.ap(), in_=BETd.ap())
            nc.sync.dma_start(out=INVCs.ap(), in_=INVCd.ap())
            nc.sync.dma_start(out=NPADs.ap(), in_=NPADd.ap())
            nc.sync.dma_start(out=CECs.ap(), in_=CECd.ap())
            nc.sync.dma_start(out=EMBHs.ap(), in_=EMBHd.ap())
            nc.gpsimd.memset(EPSs.ap(), EPS)
            nc.gpsimd.memset(HALFTPs.ap(), float(TP // 2))
            # float whose bit pattern is 0x5f3759df (fast-rsqrt magic)
            nc.gpsimd.memset(MAGICs.ap(), 13211836172961054720.0)
            # float whose bit pattern is int 1 (shift amount as int32)
            nc.gpsimd.memset(SHIFT1s.ap(), 1.401298464324817e-45)
            nc.gpsimd.memset(NHALFs.ap(), -0.5)
            nc.gpsimd.memset(THALFs.ap(), 1.5)
            nc.gpsimd.memset(INVCLs.ap()[:, 0:1], 1.0 / (KA * T))
            nc.gpsimd.memset(INVCLs.ap()[:, 1:2], 1.0 / (KB * T))
            nc.gpsimd.memset(ZBIG.ap(), 0.0)
            # zero the pad columns {0, 282, 283} of every slot
            for w in range(W):
                y3 = ysg[w].ap().rearrange("p (j t) -> p j t", j=GS)
                nc.gpsimd.tensor_copy(
                    out=y3[:, :, 0:1],
                    in_=ZBIG.ap()[:, 0:GS].rearrange("p (j o) -> p j o", o=1))
                nc.gpsimd.tensor_copy(
                    out=y3[:, :, 282:284],
                    in_=ZBIG.ap()[:, 0:2 * GS].rearrange("p (j o) -> p j o", o=2))
            for b in range(2):
                nc.gpsimd.tensor_copy(out=ysr[b].ap()[:, 0:1], in_=ZBIG.ap()[:, 0:1])
                nc.gpsimd.tensor_copy(out=ysr[b].ap()[:, 282:284], in_=ZBIG.ap()[:, 0:2])

            def evict_engine(l, w, half):
                # Returns 'v' (DVE) or 'a' (ACT). ACT takes the EARLY slabs
                # of each stream section (so its evicts finish before the
                # interleaved apply of the other stream needs ACT); DVE takes
                # the LATE slabs. Totals per layer: ~10-12 DVE, ~22-24 ACT.
                blk = 0 if w < WA else 1
                if l == 0:
                    return 'v' if (w % 2 == 0) else 'a'
                if blk == 0:
                    return 'a' if w < 4 else 'v'
                else:
                    return 'a' if (w - WA) < 7 else 'v'

            def evict_half(l, w, half, ps):
                y3 = ysg[w].ap().rearrange("p (j t) -> p j t", j=GS)
                src = ps[:].rearrange("p (j t) -> p j t", j=4)[:, :, 0:T]
                dst = y3[:, 4 * half:4 * half + 4, 1:1 + T]
                with tc.high_priority(offset=_EVOFF):
                    if evict_engine(l, w, half) == 'v':
                        nc.vector.tensor_copy(out=dst, in_=src)
                    else:
                        nc.scalar.activation(out=dst, in_=src, func=AF.Copy)

            def slab_stats(w):
                off = 0
                for i, csz in enumerate(CHUNK_SIZES):
                    c = 5 * w + i
                    nc.vector.bn_stats(out=BNSTs.ap()[:, 6 * c:6 * c + 6],
                                       in_=ysg[w].ap()[:, off:off + csz])
                    off += csz

            def ref_stats(b):
                r = BLK[b]['ref']
                nc.vector.bn_stats(out=BNSTs.ap()[:, 6 * r:6 * r + 6],
                                   in_=ysr[b].ap())

            def pre_decode(l, b):
                # partial-sum fold (DVE) -> SST[:, 2b:2b+2] -> dram ccin
                blk = BLK[b]
                c0, nch = blk['c0'], blk['nch']
                bn3 = BNSTs.ap().rearrange("p (c s) -> p c s", s=6)
                ME = bn3[:, c0:c0 + nch, 1]
                MO = bn3[:, c0:c0 + nch, 4]
                CVE = bn3[:, c0:c0 + nch, 2]
                CVO = bn3[:, c0:c0 + nch, 5]
                CEC = CECs.ap()[:, c0:c0 + nch]
                a = dA.ap()[:, 0:nch]
                bb = dB.ap()[:, 0:nch]
                cc = dC.ap()[:, 0:nch]
                nc.vector.tensor_tensor(out=a, in0=ME, in1=MO, op=OP.add)
                nc.vector.tensor_tensor(out=a, in0=a, in1=CEC, op=OP.mult)
                nc.vector.tensor_reduce(out=SST.ap()[:, 2 * b:2 * b + 1], in_=a,
                                        axis=AX.X, op=OP.add)
                nc.vector.tensor_tensor(out=bb, in0=ME, in1=ME, op=OP.mult)
                nc.vector.tensor_tensor(out=cc, in0=MO, in1=MO, op=OP.mult)
                nc.vector.tensor_tensor(out=bb, in0=bb, in1=cc, op=OP.add)
                nc.vector.tensor_tensor(out=bb, in0=bb, in1=CEC, op=OP.mult)
                nc.vector.tensor_tensor(out=bb, in0=bb, in1=CVE, op=OP.add)
                nc.vector.tensor_tensor(out=bb, in0=bb, in1=CVO, op=OP.add)
                nc.vector.tensor_reduce(out=SST.ap()[:, 2 * b + 1:2 * b + 2], in_=bb,
                                        axis=AX.X, op=OP.add)
                nc.sync.dma_start(out=cc_in[l][b].ap(), in_=SST.ap()[:, 2 * b:2 * b + 2])
                # fast-rsqrt seed from the LOCAL (this-core) variance estimate,
                # computed on DVE with no collective dependency; the Pool
                # post-decode refines it with Newton steps against the true
                # global variance.
                i32t = mybir.dt.int32
                mloc, vloc = sm[b][3], sm[b][4]
                invcl = INVCLs.ap()[:, b:b + 1]
                nc.vector.tensor_tensor(out=mloc.ap(), in0=SST.ap()[:, 2 * b:2 * b + 1],
                                        in1=invcl, op=OP.mult)
                nc.vector.tensor_tensor(out=mloc.ap(), in0=mloc.ap(), in1=mloc.ap(),
                                        op=OP.mult)
                nc.vector.tensor_tensor(out=vloc.ap(), in0=SST.ap()[:, 2 * b + 1:2 * b + 2],
                                        in1=invcl, op=OP.mult)
                nc.vector.tensor_tensor(out=vloc.ap(), in0=vloc.ap(), in1=mloc.ap(),
                                        op=OP.subtract)
                nc.vector.tensor_tensor(out=vloc.ap(), in0=vloc.ap(), in1=EPSs.ap(),
                                        op=OP.add)
                nc.vector.tensor_tensor(
                    out=mloc.ap().bitcast(i32t), in0=vloc.ap().bitcast(i32t),
                    in1=SHIFT1s.ap().bitcast(i32t), op=OP.arith_shift_right)
                nc.vector.tensor_tensor(
                    out=sm[b][7].ap().bitcast(i32t), in0=MAGICs.ap().bitcast(i32t),
                    in1=mloc.ap().bitcast(i32t), op=OP.subtract)

            def launch_cc(l, b):
                nc.gpsimd.collective_compute(
                    "AllGather", OP.bypass, replica_groups=groups4,
                    ins=[cc_in[l][b].ap()], outs=[cc_out[l][b].ap()])

            def refpad_decode(b):
                # ref-pad contribution, computed on Pool while the CC flies
                rb = 6 * BLK[b]['ref']
                MEr = BNSTs.ap()[:, rb + 1:rb + 2]
                MOr = BNSTs.ap()[:, rb + 4:rb + 5]
                CVEr = BNSTs.ap()[:, rb + 2:rb + 3]
                CVOr = BNSTs.ap()[:, rb + 5:rb + 6]
                s1r, s2r, t0 = sm[b][0], sm[b][1], sm[b][2]
                g = nc.gpsimd
                g.tensor_tensor(out=s1r.ap(), in0=MEr, in1=MOr, op=OP.add)
                g.tensor_tensor(out=s1r.ap(), in0=s1r.ap(), in1=HALFTPs.ap(),
                                op=OP.mult)
                g.tensor_tensor(out=s2r.ap(), in0=MEr, in1=MEr, op=OP.mult)
                g.tensor_tensor(out=t0.ap(), in0=MOr, in1=MOr, op=OP.mult)
                g.tensor_tensor(out=s2r.ap(), in0=s2r.ap(), in1=t0.ap(), op=OP.add)
                g.tensor_tensor(out=s2r.ap(), in0=s2r.ap(), in1=HALFTPs.ap(),
                                op=OP.mult)
                g.tensor_tensor(out=s2r.ap(), in0=s2r.ap(), in1=CVEr, op=OP.add)
                g.tensor_tensor(out=s2r.ap(), in0=s2r.ap(), in1=CVOr, op=OP.add)
                g.tensor_tensor(out=s1r.ap(), in0=s1r.ap(),
                                in1=NPADs.ap()[:, b:b + 1], op=OP.mult)
                g.tensor_tensor(out=s2r.ap(), in0=s2r.ap(),
                                in1=NPADs.ap()[:, b:b + 1], op=OP.mult)

            def post_decode(l, b):
                # gather result -> scale/shift on Pool (idle engine).
                g = nc.gpsimd
                nc.scalar.dma_start(
                    out=SG2[b].ap().rearrange("p (g s) -> p g s", g=4),
                    in_=cc_out[l][b].ap().rearrange("g p s -> p g s"))
                sg2 = SG2[b].ap().rearrange("p (g s) -> p g s", g=4)
                sg = SG[b].ap()
                g.tensor_tensor(out=sg, in0=sg2[:, 0, :], in1=sg2[:, 1, :], op=OP.add)
                g.tensor_tensor(out=sg, in0=sg, in1=sg2[:, 2, :], op=OP.add)
                g.tensor_tensor(out=sg, in0=sg, in1=sg2[:, 3, :], op=OP.add)
                s1r, s2r = sm[b][0], sm[b][1]
                g.tensor_tensor(out=sg[:, 0:1], in0=sg[:, 0:1], in1=s1r.ap(),
                                op=OP.subtract)
                g.tensor_tensor(out=sg[:, 1:2], in0=sg[:, 1:2], in1=s2r.ap(),
                                op=OP.subtract)
                meanv, msqv, varv, sdv, invv, sclv, sftv = (
                    sm[b][3], sm[b][4], sm[b][5], sm[b][6], sm[b][7], sm[b][8],
                    sm[b][9])
                invc = INVCs.ap()[:, b:b + 1]
                g.tensor_tensor(out=meanv.ap(), in0=sg[:, 0:1], in1=invc, op=OP.mult)
                g.tensor_tensor(out=msqv.ap(), in0=sg[:, 1:2], in1=invc, op=OP.mult)
                g.tensor_tensor(out=varv.ap(), in0=meanv.ap(), in1=meanv.ap(),
                                op=OP.mult)
                g.tensor_tensor(out=varv.ap(), in0=msqv.ap(), in1=varv.ap(),
                                op=OP.subtract)
                # inv = rsqrt(var + eps) via Newton on Pool, seeded from the
                # local-stats estimate (computed in pre_decode, so neither DVE
                # nor ACT has any collective-dependent instruction).
                v = sdv  # reuse scratch: v = var + eps
                g.tensor_tensor(out=v.ap(), in0=varv.ap(), in1=EPSs.ap(), op=OP.add)
                t0 = sm[b][2]
                for _ in range(3):
                    g.tensor_tensor(out=t0.ap(), in0=invv.ap(),
                                    in1=invv.ap(), op=OP.mult)
                    g.tensor_tensor(out=t0.ap(), in0=t0.ap(),
                                    in1=v.ap(), op=OP.mult)
                    g.tensor_tensor(out=t0.ap(), in0=t0.ap(),
                                    in1=NHALFs.ap(), op=OP.mult)
                    g.tensor_tensor(out=t0.ap(), in0=t0.ap(),
                                    in1=THALFs.ap(), op=OP.add)
                    g.tensor_tensor(out=invv.ap(), in0=invv.ap(),
                                    in1=t0.ap(), op=OP.mult)
                gam = GAMs.ap()[:, 2 * l + b:2 * l + b + 1]
                bet = BETs.ap()[:, 2 * l + b:2 * l + b + 1]
                g.tensor_tensor(out=sclv.ap(), in0=gam, in1=invv.ap(), op=OP.mult)
                g.tensor_tensor(out=sftv.ap(), in0=sclv.ap(), in1=meanv.ap(),
                                op=OP.mult)
                g.tensor_tensor(out=sftv.ap(), in0=bet, in1=sftv.ap(), op=OP.subtract)
                return sclv, sftv

            def apply_slab(b, w, sclv, sftv, pool=False):
                y3 = ysg[w].ap().rearrange("p (j t) -> p j t", j=GS)
                nc.scalar.activation(out=y3[:, :, 1:1 + T], in_=y3[:, :, 1:1 + T],
                                     func=AF.Gelu, bias=sftv.ap(),
                                     scale=sclv.ap())
                if pool:
                    p03 = P0.ap().rearrange("p (k o) -> p k o", o=1)
                    with tc.high_priority():
                        nc.vector.tensor_reduce(
                            out=p03[:, GS * w:GS * w + GS, :],
                            in_=y3[:, :, 1:1 + T], axis=AX.X, op=OP.add)

            def apply_ref(b, sclv, sftv):
                nc.scalar.activation(out=ysr[b].ap()[:, 1:1 + T],
                                     in_=ysr[b].ap()[:, 1:1 + T],
                                     func=AF.Gelu, bias=sftv.ap(), scale=sclv.ap())

            # ================= layers =================
            with tc.tile_pool(name="pspool", bufs=2, space="PSUM") as pspool:
                def section(l, b, pspool):
                    # Fused section for (layer l, stream b): decode the
                    # previous layer's collective (Pool), then per slab:
                    # apply(l-1) [ACT] -> conv(l) [PE] -> evict [ACT/DVE] ->
                    # bn_stats [DVE]; finally fold partial sums and launch
                    # this layer's collective.
                    _SECTIONS.append((f"conv{l}_{'AB'[b]}", nc.next_id()))
                    blk = BLK[b]
                    if l > 0:
                        sclv, sftv = post_decode(l - 1, b)
                    if True:
                        if l == 0:
                            for w in range(blk['w0'], blk['w1']):
                                pss = []
                                for half in range(2):
                                    ps = pspool.tile([128, 2048], f32, tag="ps")
                                    pss.append(ps)
                                    for j2 in range(2):
                                        smp = GS * w + 4 * half + 2 * j2
                                        x0 = xpool.tile([128, 2, TP], bf16, tag="xc0")
                                        x1 = xpool.tile([128, 2, TP], bf16, tag="xc1")
                                        x2 = xpool2.tile([45, 2, TP], bf16, tag="xc2")
                                        q = nc.sync if (j2 == 0) else nc.scalar
                                        q.dma_start(out=x0[:], in_=Xd.ap()[smp:smp + 2, 0:128, :].rearrange("b c t -> c b t"))
                                        q.dma_start(out=x1[:], in_=Xd.ap()[smp:smp + 2, 128:256, :].rearrange("b c t -> c b t"))
                                        q.dma_start(out=x2[:], in_=Xd.ap()[smp:smp + 2, 256:301, :].rearrange("b c t -> c b t"))
                                        for jj in range(2):
                                            o = 512 * (2 * j2 + jj)
                                            idx = 0
                                            for xt, base in ((x0, 0), (x1, 3)):
                                                for k in range(3):
                                                    nc.tensor.matmul(
                                                        ps[:, o:o + 282],
                                                        W1s.ap()[:, (base + k) * 128:(base + k + 1) * 128],
                                                        xt[:, jj, k:k + 282],
                                                        start=(idx == 0), stop=False)
                                                    idx += 1
                                            nc.tensor.matmul(
                                                ps[:, o:o + 282],
                                                W1s.ap()[0:45, 6 * 128:7 * 128],
                                                x2[0:45, jj, 1:283], start=False, stop=True)
                                    evict_half(l, w, half, ps)
                                slab_stats(w)
                            # ref slot: conv(0) == 0
                            nc.scalar.activation(out=ysr[b].ap()[:, 1:1 + T],
                                                 in_=ZBIG.ap()[:, 0:T], func=AF.Copy)
                            ref_stats(b)
                        else:
                            w0 = (l - 1) * 3
                            apply_slab(b, blk['w0'], sclv, sftv)
                            for w in range(blk['w0'], blk['w1']):
                                pss = []
                                for half in range(2):
                                    ps = pspool.tile([128, 2048], f32, tag="ps")
                                    pss.append(ps)
                                    for j4 in range(4):
                                        j = 4 * half + j4
                                        o = 512 * j4
                                        for k in range(3):
                                            nc.tensor.matmul(
                                                ps[:, o:o + 282],
                                                WRs.ap()[:, (w0 + k) * 128:(w0 + k + 1) * 128],
                                                ysg[w].ap()[:, j * TP + k:j * TP + k + 282],
                                                start=(k == 0), stop=(k == 2))
                                    evict_half(l, w, half, ps)
                                    if half == 0 and w + 1 < blk['w1']:
                                        apply_slab(b, w + 1, sclv, sftv)
                                slab_stats(w)
                            # ref slot conv
                            apply_ref(b, sclv, sftv)
                            psr = pspool.tile([128, 2048], f32, tag="ps")
                            for k in range(3):
                                nc.tensor.matmul(
                                    psr[:, 0:282],
                                    WRs.ap()[:, (w0 + k) * 128:(w0 + k + 1) * 128],
                                    ysr[b].ap()[:, k:k + 282],
                                    start=(k == 0), stop=(k == 2))
                            nc.scalar.activation(out=ysr[b].ap()[:, 1:1 + T],
                                                 in_=psr[:, 0:T], func=AF.Copy)
                            ref_stats(b)
                        pre_decode(l, b)
                        launch_cc(l, b)
                        refpad_decode(b)

                def final_apply(b):
                    _SECTIONS.append((f"apply8_{'AB'[b]}", nc.next_id()))
                    blk = BLK[b]
                    sclv, sftv = post_decode(8, b)
                    for w in range(blk['w0'], blk['w1']):
                        apply_slab(b, w, sclv, sftv, pool=True)
                    apply_ref(b, sclv, sftv)

                for l in range(9):
                    section(l, 0, pspool)
                    section(l, 1, pspool)
                final_apply(0)
                final_apply(1)
                nc.vector.tensor_copy(out=P0h.ap(), in_=P0.ap())

            _SECTIONS.append(("head", nc.next_id()))
            # ================= head =================
            with tc.tile_pool(name="hppool", bufs=8, space="PSUM") as hppool:
                for n in range(NCHUNKS):
                    rows = min(128, NCLS - n * 128)
                    hp = hppool.tile([128, K], f32, tag="hp")
                    nc.tensor.matmul(hp[:], EMBHs.ap()[:, n * 128:(n + 1) * 128],
                                     P0h.ap(), start=True, stop=True)
                    hc = hcpool.tile([128, K], f32, tag="hc")
                    nc.scalar.activation(out=hc[:], in_=hp[:], func=AF.Copy)
                    nc.sync.dma_start(out=OUTd.ap()[n * 128:n * 128 + rows, :],
                                      in_=hc[0:rows, :])

    nc.finalize()
    return nc


def kernel(**inputs):
    from concourse.bass_utils import run_bass_kernel_spmd
    import ml_dtypes

    X = np.asarray(inputs["X"], dtype=np.float32)
    w1_0 = np.asarray(inputs["w1_0"], dtype=np.float32)
    w_rest = np.asarray(inputs["w_rest"], dtype=np.float32)
    gammas = np.asarray(inputs["gammas"], dtype=np.float32)
    betas = np.asarray(inputs["betas"], dtype=np.float32)
    emb = np.asarray(inputs["emb"], dtype=np.float32)
    headW = np.asarray(inputs["headW"], dtype=np.float32)
    headb = np.asarray(inputs["headb"], dtype=np.float32)
    sidx = np.asarray(inputs["subject_idxs"]).astype(np.int64)

    B = X.shape[0]
    counts = np.bincount(sidx, minlength=S)
    order = np.argsort(sidx, kind="stable")
    offs = np.zeros(S + 1, np.int64)
    offs[1:] = np.cumsum(counts)

    def r8(x):
        return ((int(x) + GS - 1) // GS) * GS

    # A-stream: two largest subjects; B-stream: two smallest.
    desc = list(np.argsort(-counts, kind="stable"))
    subjA = [int(desc[0]), int(desc[1])]   # group 0, group 1
    subjB = [int(desc[2]), int(desc[3])]
    KA = r8(max(GS, -(-counts[subjA[0]] // 4), -(-counts[subjA[1]] // 4)))
    KB = r8(max(GS, -(-counts[subjB[0]] // 4), -(-counts[subjB[1]] // 4)))
    K = KA + KB

    quarters = {}
    for s in range(S):
        ids = order[offs[s]:offs[s + 1]]
        quarters[s] = np.array_split(ids, 4)

    # ---- shared host-side weight prep ----
    W1p = np.zeros((128, 7 * 128), np.float32)
    for c in range(2):
        for k in range(3):
            W1p[:, (c * 3 + k) * 128:(c * 3 + k + 1) * 128] = \
                w1_0[:, c * 128:(c + 1) * 128, k].T
    for k in range(3):
        W1p[15 * k:15 * k + 15, 6 * 128:7 * 128] = w1_0[:, 256:271, k].T
    WRp = np.zeros((128, 24 * 128), np.float32)
    eye = np.eye(H, dtype=np.float32)
    for l in range(8):
        for k in range(3):
            wt = w_rest[l, :, :, k].T.copy()
            if k == 1:
                wt += eye
            WRp[:, (l * 3 + k) * 128:(l * 3 + k + 1) * 128] = wt
    EMBHp = np.zeros((128, NCHUNKS * 128), np.float32)
    EMBHp[:, 0:NCLS] = headW[:, 0:H].T / float(T)
    WAn = KA // GS
    WBn = KB // GS
    CECp = np.tile(np.array([[c // 2 for c in CHUNK_SIZES]], np.float32),
                   (128, WAn + WBn)).astype(np.float32)

    in_maps = []
    core_ids_list = []
    for c in range(N_CORES):
        g = c // 4
        q = c % 4
        sA, sB = subjA[g], subjB[g]
        idsA = quarters[sA][q]
        idsB = quarters[sB][q]
        core_ids_list.append((idsA, idsB))
        Xc = np.zeros((K, 301, TP), np.float32)
        for ids, base in ((idsA, 0), (idsB, KA)):
            n = len(ids)
            if n:
                Xc[base:base + n, 0:CIN, 1:1 + T] = X[ids]
                # pre-shifted 15-channel tail bands
                xt = Xc[base:base + n, 256:CIN, :].copy()
                Xc[base:base + n, 256:271, 1:] = xt[:, :, :-1]
                Xc[base:base + n, 256:271, 0] = 0.0
                Xc[base:base + n, 271:286, :] = xt
                Xc[base:base + n, 286:301, :-1] = xt[:, :, 1:]
                Xc[base:base + n, 286:301, -1] = 0.0
        INVC = np.zeros((128, 2), np.float32)
        INVC[:, 0] = 1.0 / (max(int(counts[sA]), 1) * T)
        INVC[:, 1] = 1.0 / (max(int(counts[sB]), 1) * T)
        NPAD = np.zeros((128, 2), np.float32)
        NPAD[:, 0] = float(4 * KA - int(counts[sA]))
        NPAD[:, 1] = float(4 * KB - int(counts[sB]))
        GAMp = np.zeros((128, 18), np.float32)
        BETp = np.zeros((128, 18), np.float32)
        for l in range(9):
            GAMp[:, 2 * l + 0] = gammas[l, sA, :]
            GAMp[:, 2 * l + 1] = gammas[l, sB, :]
            BETp[:, 2 * l + 0] = betas[l, sA, :]
            BETp[:, 2 * l + 1] = betas[l, sB, :]
        in_maps.append({
            "Xd": Xc.astype(ml_dtypes.bfloat16),
            "W1d": W1p.astype(ml_dtypes.bfloat16),
            "WRd": WRp,
            "GAMd": GAMp,
            "BETd": BETp,
            "INVCd": INVC,
            "NPADd": NPAD,
            "CECd": CECp,
            "EMBHd": EMBHp.astype(ml_dtypes.bfloat16),
        })

    key = (KA, KB)
    if key not in _CACHE:
        _CACHE[key] = _build(KA, KB)
    nc = _CACHE[key]

    res = run_bass_kernel_spmd(nc, in_maps, core_ids=list(range(N_CORES)))
    kernel.last_results = res

    out = np.zeros((B, NCLS), np.float32)
    b2 = emb @ headW[:, H:].T + headb[None, :]  # [S, NCLS]
    for c in range(N_CORES):
        g = c // 4
        idsA, idsB = core_ids_list[c]
        resc = res.results[c]["OUTd"].T  # [K, NCLS]
        if len(idsA):
            out[idsA] = resc[0:len(idsA)] + b2[subjA[g]][None, :]
        if len(idsB):
            out[idsB] = resc[KA:KA + len(idsB)] + b2[subjB[g]][None, :]
    return out
